# revision 28
# baseline (speedup 1.0000x reference)
"""Trainium2 Bass kernel for nn_Attention_4449586119407.

GQA attention layer (B=2, L=2048, D=2048, 32 Q heads / 8 KV heads, RoPE,
causal) sharded over 8 NeuronCores: data-parallel over batch (2) x
tensor-parallel over heads (4 groups of 8 Q heads / 2 KV heads).
wq/wk/wv column-sharded, wo row-sharded; the wo partial sums are reduced
on the host.

Device-side layout (per core):
  - All matmuls contract over the SBUF partition dim.  x is pre-transposed
    on the host (xT [D, L]) so QKV projections produce channel-major
    qT/kT [c, l] directly.
  - RoPE: wq/wk rows are permuted on the host per 128-row pair block as
    [te_h0(32) | te_h1(32) | to_h0(32) | to_h1(32)], so the rotation
    partner lives exactly 64 partitions away.  The rotation is then pure
    DVE work with partition-offset operands (no SBUF-SBUF DMA):
        dest[0:64]   = ps[0:64]*cos  - ps[64:128]*sin
        dest[64:128] = ps[64:128]*cos + ps[0:64]*sin
    k-RoPE writes its quarter-blocks directly into the zero-padded kz
    stationary tiles.
  - Scores are computed transposed (S[j, i] = k . q) so the softmaxed tile
    feeds the P@V matmul directly as the stationary operand.  Softmax skips
    max-subtraction; the denominator comes free as a "ones" column of V.
  - Causality is structural: only lower-triangle j-blocks are computed;
    diagonal blocks are masked AFTER the exp by an f16 0/1 triangle
    multiply (keeps the PSUM score bank live for the minimum time).
  - The attention inner loop is software-pipelined: the P@V matmul for
    block jb issues two iterations after its scores so the Scalar-engine
    exp latency never stalls the PE.
"""

import numpy as np

B, L, D = 2, 2048, 2048
NH, NKV, HD = 32, 8, 64
SCALE = HD ** -0.5
NCORES = 8
F32 = np.float32

_CACHE = {}


def _build_nc():
    from contextlib import ExitStack

    import concourse.tile as tile
    from concourse import bacc, mybir

    f32 = mybir.dt.float32
    f16 = mybir.dt.float16
    AF = mybir.ActivationFunctionType
    ALU = mybir.AluOpType

    nc = bacc.Bacc("TRN2", target_bir_lowering=False, debug=False,
                   num_devices=NCORES)

    xT = nc.dram_tensor("xT", [D, L], f16, kind="ExternalInput").ap()
    wq_sb_d = nc.dram_tensor("wq_sb", [128, 8192], f16, kind="ExternalInput").ap()
    wk_sb_d = nc.dram_tensor("wk_sb", [128, 2048], f16, kind="ExternalInput").ap()
    wv_sb_d = nc.dram_tensor("wv_sb", [128, 2048], f16, kind="ExternalInput").ap()
    wo_sb_d = nc.dram_tensor("wo_sb", [128, 8192], f16, kind="ExternalInput").ap()
    csin_d = nc.dram_tensor("csin", [128, 4096], f32, kind="ExternalInput").ap()
    tri_d = nc.dram_tensor("tri", [128, 128], f16, kind="ExternalInput").ap()
    ident_d = nc.dram_tensor("ident", [128, 128], f32, kind="ExternalInput").ap()
    y_d = nc.dram_tensor("y", [L, D], f16, kind="ExternalOutput").ap()

    with tile.TileContext(nc) as tc:
        with ExitStack() as ctx:
            singles = ctx.enter_context(tc.tile_pool(name="singles", bufs=1))
            xt0_p = ctx.enter_context(tc.tile_pool(name="xt0", bufs=8))
            xtb_p = ctx.enter_context(tc.tile_pool(name="xtb", bufs=2))
            maps_p = ctx.enter_context(tc.tile_pool(name="maps", bufs=2))
            qrot_p = ctx.enter_context(tc.tile_pool(name="qrot", bufs=8))
            ptile_p = ctx.enter_context(tc.tile_pool(name="pt", bufs=4))
            tmp_p = ctx.enter_context(tc.tile_pool(name="tmp", bufs=3))
            attT_p = ctx.enter_context(tc.tile_pool(name="attT", bufs=9))
            nrm_p = ctx.enter_context(tc.tile_pool(name="nrm", bufs=2))
            ysb_p = ctx.enter_context(tc.tile_pool(name="ysb", bufs=3))
            pj_ps = ctx.enter_context(tc.tile_pool(name="pj", bufs=2, space="PSUM"))
            sc_ps = ctx.enter_context(tc.tile_pool(name="sc", bufs=2, space="PSUM"))
            at_ps = ctx.enter_context(tc.tile_pool(name="at", bufs=2, space="PSUM"))

            # ---- warmup: keep PE busy while the first DMAs land ----
            warm_sb = singles.tile([128, 128], f16)
            nc.vector.memset(warm_sb[:], 0.0)
            warm_ps = pj_ps.tile([64, 64], f32, tag="pj", name="warm_ps")
            for _ in range(24):
                nc.tensor.matmul(warm_ps[:], warm_sb[:, 0:64], warm_sb[:, 0:64],
                                 start=True, stop=True)

            # ---- resident constants ----
            wq_sb = singles.tile([128, 8192], f16)
            wk_sb = singles.tile([128, 2048], f16)
            wv_sb = singles.tile([128, 2048], f16)
            tri_sb = singles.tile([128, 128], f16)
            ident_sb = singles.tile([128, 128], f32)
            wo_sb = singles.tile([128, 8192], f16)
            # kz[kv][e]: rotated k for kv head, at q-head-e's partition sets
            # ({32e:32e+32} and {64+32e:96+32e}), zeros elsewhere -> K=128
            # scores matmuls with full partitions.
            kz = [[singles.tile([128, 2048], f16, name=f"kz{kv}{e}")
                   for e in range(2)] for kv in range(2)]
            for kv in range(2):
                for e in range(2):
                    nc.gpsimd.memset(kz[kv][e][:], 0.0)
            # per jb: [ones(64) | v(64)] -> full 128-col stationary; the ones
            # column at offset 0 puts the softmax denominator on PSUM
            # partition 0, where reciprocal_approx_fast can read it directly.
            vext = [singles.tile([128, 2048], f16, name=f"vext{kv}")
                    for kv in range(2)]
            for kv in range(2):
                nc.gpsimd.memset(vext[kv][:], 1.0)

            def diag_off(jb, lc):
                od = 128 * jb - 512 * lc
                return od if 0 <= od < 512 else None

            q_tiles = {}

            # ---------------- loads ----------------
            xT_r = xT.rearrange("(db p) l -> p db l", p=128)

            def load_xt(lc):
                lsl = slice(lc * 512, (lc + 1) * 512)
                xt = xtb_p.tile([128, 8192], f16, tag="xtb", name="xtb")
                nc.sync.dma_start(xt.rearrange("p (db c) -> p db c", db=16),
                                  xT_r[:, :, lsl])
                cs = maps_p.tile([128, 1024], f32, tag="cs", name="cs")
                nc.sync.dma_start(cs[:], csin_d[:, lc * 1024:(lc + 1) * 1024])
                return xt, cs

            # ---------------- projections + rope ----------------
            # The BIR verifier requires all SBUF inputs of a tensor_tensor to
            # share a start partition; PSUM inputs are exempt.  So: sin
            # product to SBUF first, then scale ps by cos IN PLACE in PSUM —
            # each rotation half-op then has one PSUM + one SBUF input.
            def rope_q(ps, dest, cs):
                t2 = tmp_p.tile([128, 512], f32, tag="t2", bufs=2, name="t2")
                nc.vector.tensor_tensor(t2[:], ps[:], cs[:, 512:1024], op=ALU.mult)
                nc.vector.tensor_tensor(ps[:], ps[:], cs[:, 0:512], op=ALU.mult)
                nc.vector.tensor_tensor(dest[0:64, :], ps[0:64, :],
                                        t2[64:128, :], op=ALU.subtract)
                nc.vector.tensor_tensor(dest[64:128, :], ps[64:128, :],
                                        t2[0:64, :], op=ALU.add)

            def rope_k(ps, lc, cs):
                lsl = slice(lc * 512, (lc + 1) * 512)
                t2 = tmp_p.tile([128, 512], f32, tag="t2", bufs=2, name="t2")
                nc.vector.tensor_tensor(t2[:], ps[:], cs[:, 512:1024], op=ALU.mult)
                nc.vector.tensor_tensor(ps[:], ps[:], cs[:, 0:512], op=ALU.mult)
                for kv in range(2):
                    for e in range(2):
                        nc.vector.tensor_tensor(
                            kz[kv][e][32 * e:32 * e + 32, lsl],
                            ps[32 * kv:32 * kv + 32, :],
                            t2[64 + 32 * kv:96 + 32 * kv, :], op=ALU.subtract)
                        nc.vector.tensor_tensor(
                            kz[kv][e][64 + 32 * e:96 + 32 * e, lsl],
                            ps[64 + 32 * kv:96 + 32 * kv, :],
                            t2[32 * kv:32 * kv + 32, :], op=ALU.add)

            def proj_q(lc, cb, xs, cs):
                ps = pj_ps.tile([128, 512], f32, tag="pj", name="ps_q")
                for db in range(16):
                    nc.tensor.matmul(
                        ps[:], wq_sb[:, (cb * 16 + db) * 128:(cb * 16 + db + 1) * 128],
                        xs(db), start=(db == 0), stop=(db == 15))
                qt = qrot_p.tile([128, 512], f16, name="qt")
                rope_q(ps, qt, cs)
                q_tiles[(cb, lc)] = qt

            def proj_k(lc, xs, cs):
                ps = pj_ps.tile([128, 512], f32, tag="pj", name="ps_k")
                for db in range(16):
                    nc.tensor.matmul(
                        ps[:], wk_sb[:, db * 128:(db + 1) * 128],
                        xs(db), start=(db == 0), stop=(db == 15))
                rope_k(ps, lc, cs)

            def proj_v(lc, xs):
                vt = tmp_p.tile([128, 512], f32, tag="vt", bufs=2, name="vt")
                ps = pj_ps.tile([128, 512], f32, tag="pj", name="ps_v")
                for db in range(16):
                    nc.tensor.matmul(
                        ps[:], wv_sb[:, db * 128:(db + 1) * 128],
                        xs(db), start=(db == 0), stop=(db == 15))
                nc.scalar.copy(vt[:], ps[:])
                for j in range(4):
                    jb = 4 * lc + j
                    ps = pj_ps.tile([128, 128], f32, tag="pj", name="ps_t")
                    nc.tensor.transpose(ps[:], vt[:, j * 128:(j + 1) * 128],
                                        ident_sb[:])
                    for kv in range(2):
                        nc.vector.tensor_copy(
                            vext[kv][:, jb * 128 + 64:(jb + 1) * 128],
                            ps[:, kv * 64:kv * 64 + 64])

            def do_proj(lc, loaded):
                xt, cs = loaded
                xs = lambda db: xt[:, db * 512:(db + 1) * 512]
                proj_q(lc, 0, xs, cs)
                proj_k(lc, xs, cs)
                proj_v(lc, xs)
                for cb in range(1, 4):
                    proj_q(lc, cb, xs, cs)

            # ---------------- attention ----------------
            def do_att(lc, pairs=(0, 1, 2, 3)):
                njb = 4 * lc + 4
                for t in pairs:
                    heads = (2 * t, 2 * t + 1)
                    aps_ = [at_ps.tile([128, 512], f32, tag="at", name=f"at{e}")
                            for e in range(2)]
                    live = {}

                    def scores_step(jb):
                        o = max(0, 128 * jb - 512 * lc)
                        S = sc_ps.tile([128, 1024], f32, tag="sc", name="S")
                        for e, h in enumerate(heads):
                            kt = kz[h // 4][h % 2]
                            nc.tensor.matmul(
                                S[:, e * 512 + o:(e + 1) * 512],
                                kt[:, jb * 128:(jb + 1) * 128],
                                q_tiles[(h // 2, lc)][:, o:512],
                                start=True, stop=True)
                        P = ptile_p.tile([128, 1024], f16, name="P")
                        if o == 0:
                            nc.scalar.activation(P[:], S[:], AF.Exp)
                        else:
                            s3 = S.rearrange("p (e c) -> p e c", e=2)[:, :, o:512]
                            p3 = P.rearrange("p (e c) -> p e c", e=2)[:, :, o:512]
                            nc.scalar.activation(p3, s3, AF.Exp)
                        od = diag_off(jb, lc)
                        if od is not None:
                            p3 = P.rearrange("p (e c) -> p e c", e=2)[:, :, od:od + 128]
                            t3m = tri_sb[:].unsqueeze(1).broadcast_to([128, 2, 128])
                            nc.vector.tensor_tensor(p3, p3, t3m, op=ALU.mult)
                        live[jb] = (P, o)

                    def pv_step(jb):
                        P, o = live.pop(jb)
                        for e, h in enumerate(heads):
                            kv = h // 4
                            nc.tensor.matmul(
                                aps_[e][:, o:512],
                                vext[kv][:, jb * 128:jb * 128 + 128],
                                P[:, e * 512 + o:(e + 1) * 512],
                                start=(jb == 0), stop=(jb == njb - 1),
                                skip_group_check=True)

                    for jb in range(njb):
                        scores_step(jb)
                        if jb >= 2:
                            pv_step(jb - 2)
                    pv_step(njb - 2)
                    pv_step(njb - 1)

                    attT = attT_p.tile([128, 512], f16, name="attT")
                    q_tiles[("attT", lc, t)] = attT
                    for e in range(2):
                        # den sits on PSUM partition 0 (ones col 0 of vext):
                        # recip reads it straight from PSUM; the normalize
                        # multiply reads the numerator rows from PSUM too
                        # (no staging copies).
                        rcpb = nrm_p.tile([64, 512], f32, tag="rcpb", bufs=2,
                                          name="rcpb")
                        nc.vector.reciprocal_approx_fast(out=rcpb[0:1, :],
                                                         in_=aps_[e][0:1, :])
                        nc.gpsimd.partition_broadcast(rcpb[:], rcpb[0:1, :])
                        nc.vector.tensor_tensor(
                            attT[64 * e:64 * e + 64, :], aps_[e][64:128, :],
                            rcpb[:], op=ALU.mult)

            # ---------------- output projection ----------------
            def do_outproj(lc, lbs=(0, 1, 2, 3), final=False):
                for lb in lbs:
                    ysb = ysb_p.tile([128, 2048], f16, name="ysb")
                    rows = slice(lc * 512 + lb * 128, lc * 512 + (lb + 1) * 128)
                    for mc in range(4):
                        ps = pj_ps.tile([128, 512], f32, tag="pj", name="ps_o")
                        for cb in range(4):
                            nc.tensor.matmul(
                                ps[:],
                                q_tiles[("attT", lc, cb)][:, lb * 128:(lb + 1) * 128],
                                wo_sb[:, cb * 2048 + mc * 512:cb * 2048 + (mc + 1) * 512],
                                start=(cb == 0), stop=(cb == 3))
                        if mc % 2 == 0:
                            nc.scalar.copy(ysb[:, mc * 512:(mc + 1) * 512], ps[:])
                        else:
                            nc.vector.tensor_copy(ysb[:, mc * 512:(mc + 1) * 512],
                                                  ps[:])
                        if final:
                            nc.gpsimd.dma_start(
                                y_d[rows, mc * 512:(mc + 1) * 512],
                                ysb[:, mc * 512:(mc + 1) * 512])
                    if not final:
                        nc.gpsimd.dma_start(y_d[rows, :], ysb[:])

            # ---------------- schedule ----------------
            # chunk-0: 4 groups of 4 db-blocks, interleaved with the wq
            # column chunks they unblock.
            xt0 = []
            nc.sync.dma_start(wk_sb[:, 0:1024], wk_sb_d[:, 0:1024])
            nc.sync.dma_start(wq_sb[:, 0:512], wq_sb_d[:, 0:512])
            for g in range(8):
                t = xt0_p.tile([128, 1024], f16, tag="xt0", name=f"xt0g{g}")
                nc.sync.dma_start(t.rearrange("p (db c) -> p db c", db=2),
                                  xT_r[:, 2 * g:2 * g + 2, 0:512])
                xt0.append(t)
                if g == 0:
                    nc.sync.dma_start(wk_sb[:, 1024:2048], wk_sb_d[:, 1024:2048])
                elif g == 1:
                    nc.sync.dma_start(wq_sb[:, 512:1024], wq_sb_d[:, 512:1024])
                elif g == 2:
                    cs0 = maps_p.tile([128, 1024], f32, tag="cs", name="cs")
                    nc.sync.dma_start(cs0[:], csin_d[:, 0:1024])
                elif g == 3:
                    nc.sync.dma_start(wq_sb[:, 1024:2048], wq_sb_d[:, 1024:2048])
            nc.sync.dma_start(wv_sb[:], wv_sb_d[:])
            nc.sync.dma_start(tri_sb[:], tri_d[:])
            nc.sync.dma_start(ident_sb[:], ident_d[:])
            for cb in range(1, 4):
                nc.sync.dma_start(wq_sb[:, cb * 2048:(cb + 1) * 2048],
                                  wq_sb_d[:, cb * 2048:(cb + 1) * 2048])

            xs0 = lambda db: xt0[db // 2][:, (db % 2) * 512:(db % 2 + 1) * 512]
            proj_k(0, xs0, cs0)
            proj_q(0, 0, xs0, cs0)
            proj_v(0, xs0)
            do_att(0, pairs=(0,))
            proj_q(0, 1, xs0, cs0)
            do_att(0, pairs=(1,))
            ld = load_xt(1)
            nc.sync.dma_start(wo_sb[:], wo_sb_d[:])
            proj_q(0, 2, xs0, cs0)
            do_att(0, pairs=(2,))
            proj_q(0, 3, xs0, cs0)
            do_att(0, pairs=(3,))

            do_proj(1, ld)
            ld = load_xt(2)
            do_att(1, pairs=(0, 1))
            do_outproj(0, lbs=(0, 1))
            do_att(1, pairs=(2, 3))
            do_outproj(0, lbs=(2, 3))

            do_proj(2, ld)
            ld = load_xt(3)
            do_att(2, pairs=(0, 1))
            do_outproj(1, lbs=(0, 1))
            do_att(2, pairs=(2, 3))
            do_outproj(1, lbs=(2, 3))

            do_proj(3, ld)
            do_att(3, pairs=(0, 1))
            do_outproj(2, lbs=(0, 1))
            do_att(3, pairs=(2, 3))
            do_outproj(2, lbs=(2, 3))
            do_outproj(3, final=True)

    nc.compile()
    return nc


def _prep_core_inputs(core, x, wq, wk, wv, wo, fc, fs, mask, xT_cache):
    b, g = divmod(core, 4)
    hq0 = 8 * g
    if b not in xT_cache:
        xT_cache[b] = np.ascontiguousarray(x[b].T, dtype=np.float16)
    xT = xT_cache[b]

    def pair_perm(w2h):
        # w2h: [128, D] rows = [h0 (64) | h1 (64)] -> pair-interleaved rope
        # layout [h0 even (32) | h1 even (32) | h0 odd (32) | h1 odd (32)]
        return np.concatenate([w2h[0:64][0::2], w2h[64:128][0::2],
                               w2h[0:64][1::2], w2h[64:128][1::2]], axis=0)

    wq_s = (wq[hq0 * 64:(hq0 + 8) * 64] * SCALE).astype(F32)
    wq_p = np.concatenate(
        [pair_perm(wq_s[p * 128:(p + 1) * 128]) for p in range(4)], 0)
    wqT = wq_p.T  # [D, 512]
    # wq_sb[p, (cb*16+db)*128 + c] = wqT[db*128+p, cb*128+c]
    wq_sb = np.ascontiguousarray(
        wqT.reshape(16, 128, 4, 128).transpose(1, 2, 0, 3).reshape(128, 8192),
        dtype=np.float16)

    wk_s = wk[2 * g * 64:(2 * g + 2) * 64]
    wk_p = pair_perm(wk_s)
    wkT = wk_p.T  # [D, 128]
    wk_sb = np.ascontiguousarray(
        wkT.reshape(16, 128, 128).transpose(1, 0, 2).reshape(128, 2048),
        dtype=np.float16)

    wvT = wv[2 * g * 64:(2 * g + 2) * 64].T  # [D, 128]
    wv_sb = np.ascontiguousarray(
        wvT.reshape(16, 128, 128).transpose(1, 0, 2).reshape(128, 2048),
        dtype=np.float16)

    woT = wo[:, hq0 * 64:(hq0 + 8) * 64].T  # [512, D]
    wo_sb = np.ascontiguousarray(
        woT.reshape(4, 128, 4, 512).transpose(1, 0, 2, 3).reshape(128, 8192),
        dtype=np.float16)

    cosT = np.tile(fc.T, (4, 1)).astype(F32)  # [128, L]
    sinT = np.tile(fs.T, (4, 1)).astype(F32)
    csin = np.empty((128, 4096), F32)
    for lc in range(4):
        csin[:, lc * 1024:lc * 1024 + 512] = cosT[:, lc * 512:(lc + 1) * 512]
        csin[:, lc * 1024 + 512:(lc + 1) * 1024] = sinT[:, lc * 512:(lc + 1) * 512]

    # tri[j, i] = 1 where i >= j (value kept after exp on diagonal blocks)
    tri = np.triu(np.ones((128, 128), np.float16))

    return {"xT": xT, "wq_sb": wq_sb, "wk_sb": wk_sb, "wv_sb": wv_sb,
            "wo_sb": wo_sb, "csin": np.ascontiguousarray(csin),
            "tri": np.ascontiguousarray(tri),
            "ident": np.eye(128, dtype=F32)}


def kernel(x, wq, wk, wv, wo, freqs_cos, freqs_sin, mask):
    from concourse import bass_utils

    if "nc" not in _CACHE:
        _CACHE["nc"] = _build_nc()
    nc = _CACHE["nc"]

    x = np.asarray(x, F32)
    xT_cache = {}
    in_maps = [
        _prep_core_inputs(c, x, np.asarray(wq, F32), np.asarray(wk, F32),
                          np.asarray(wv, F32), np.asarray(wo, F32),
                          np.asarray(freqs_cos, F32), np.asarray(freqs_sin, F32),
                          np.asarray(mask, F32), xT_cache)
        for c in range(NCORES)
    ]
    res = bass_utils.run_bass_kernel_spmd(nc, in_maps, core_ids=list(range(NCORES)))
    out = np.zeros((B, L, D), F32)
    for c in range(NCORES):
        out[c // 4] += res.results[c]["y"].astype(F32)
    return out
